# revision 7
# baseline (speedup 1.0000x reference)
"""CrossModalityAttention Trainium2 Bass kernel.

Data-parallel over batch: 8 cores, one batch element each.
Per core (b): out[b] = softmax((img[b]@Wq + bq) @ (txt[b]@Wk + bk)^T / 32) @ (txt[b]@Wv + bv)

Key choices vs the fp32r baseline (545us):
  * All matmul operands bf16 (host casts img/txt/W* to bf16). fp32 PSUM
    accumulation. Measured end-to-end rel err ~3e-3 << 2e-2 budget.
  * bk dropped entirely: S[q,k] = Q.K[k] + Q.bk is a per-row constant shift
    under row-softmax, so it cancels.
  * imgT / txtT produced by XBAR DMA transpose (16x128-tile crossbar, 2-byte
    dtypes) straight from DRAM -- zero PE transpose work, no identity matrix.
  * Everything SBUF-resident: txtT 3MB, V 4MB, Kt 4MB, weights 5MB, per-qc
    Qt/imgT/E double-buffered. No DRAM scratch round-trips.
  * Phase order V -> Kt -> per-q-chunk [Qt -> S -> exp -> O -> epilogue],
    emitted so the PE never waits on anything but the first ~2MB of DMA.

Layout (contraction dim always = partition dim):
  txtT[d, k] (XBAR)      imgT[i, q-chunk] (XBAR, double-buffered)
  V[k, h]    = txtT^T Wv               -> SBUF bf16 (bv folded in epilogue)
  Kt[h, k]   = Wk^T txtT               -> SBUF bf16 (no bk)
  Qt[h, q]   = Wq^T imgT + bq          -> SBUF bf16, per q-chunk
  S[k, q]    = Kt^T Qt   (psum f32)
  E = exp(S/32)          (ACT, psum -> SBUF bf16)
  O[q, h]    = E^T V, sums[q] = E^T ones, out = O/sums + bv
"""

import numpy as np
import ml_dtypes

import concourse.bass as bass
import concourse.tile as tile
from concourse import bacc, mybir
from concourse.bass_utils import run_bass_kernel_spmd

F32 = mybir.dt.float32
BF16 = mybir.dt.bfloat16
AF = mybir.ActivationFunctionType

P = 128
B, LQ, LK = 8, 2048, 2048
IMG, TXT, HID = 1024, 768, 1024
NKT = LK // P                 # 16 key tiles
NTC = TXT // P                # 6 txt contraction chunks
NIC = IMG // P                # 8 img contraction chunks
NHT = HID // P                # 8 hid tiles
QC = 512                      # q chunk width
NQC = LQ // QC                # 4
SCALE = 1.0 / np.sqrt(np.float32(HID))

_CACHED = {}


def build_kernel(reps=1):
    nc = bacc.Bacc("TRN2", target_bir_lowering=False, debug=False)
    img = nc.dram_tensor("img", [LQ, IMG], BF16, kind="ExternalInput").ap()
    txt = nc.dram_tensor("txt", [LK, TXT], BF16, kind="ExternalInput").ap()
    wq = nc.dram_tensor("wq", [IMG, HID], BF16, kind="ExternalInput").ap()
    wk = nc.dram_tensor("wk", [TXT, HID], BF16, kind="ExternalInput").ap()
    wv = nc.dram_tensor("wv", [TXT, HID], BF16, kind="ExternalInput").ap()
    bq = nc.dram_tensor("bq", [HID], F32, kind="ExternalInput").ap()
    bv = nc.dram_tensor("bv", [HID], F32, kind="ExternalInput").ap()
    out = nc.dram_tensor("out_attn", [LQ, HID], F32, kind="ExternalOutput").ap()

    with tile.TileContext(nc) as tc:
        with (
            tc.tile_pool(name="sb", bufs=1) as sb,
            tc.tile_pool(name="psum", bufs=1, space="PSUM") as psum,
        ):
            ones = sb.tile([P, 2], BF16, tag="ones")
            nc.vector.memset(ones[:], 1.0)
            bq_t = sb.tile([P, NHT], F32, tag="bq")
            nc.gpsimd.dma_start(out=bq_t[:], in_=bq.rearrange("(t p) -> p t", p=P))
            bv_bc = sb.tile([P, HID], F32, tag="bv")
            nc.gpsimd.dma_start(out=bv_bc[:], in_=bv.partition_broadcast(P))

            v_t = [sb.tile([P, HID], BF16, tag=f"v{k}", name=f"v{k}")
                   for k in range(NKT)]
            kt_t = [sb.tile([P, LK], BF16, tag=f"kt{h}", name=f"kt{h}")
                    for h in range(NHT)]

            for rep in range(reps):
                with tc.tile_pool(name=f"proj{rep}", bufs=1) as proj:
                    # ---- weight loads on Pool queue (parallel to SP) ------
                    # wv split in h-halves so V(k=0,hc=0) gates on 0.75MB only
                    wv_all = proj.tile([P, NTC, HID], BF16, tag="wv", name="wv")
                    wv_r = wv.rearrange("(c p) h -> p c h", p=P)
                    nc.gpsimd.dma_start(out=wv_all[:, :, 0:QC],
                                      in_=wv_r[:, :, 0:QC])
                    nc.gpsimd.dma_start(out=wv_all[:, :, QC:HID],
                                      in_=wv_r[:, :, QC:HID])
                    wk_all = proj.tile([P, NTC, HID], BF16, tag="wk", name="wk")
                    nc.gpsimd.dma_start(out=wk_all[:],
                                      in_=wk.rearrange("(c p) h -> p c h", p=P))
                    wq_all = sb.tile([P, NIC, HID], BF16, tag="wq", name="wq")
                    nc.gpsimd.dma_start(out=wq_all[:],
                                      in_=wq.rearrange("(c p) h -> p c h", p=P))

                    # ---- txtT via XBAR transpose on SP queue --------------
                    txtT = [proj.tile([P, LK], BF16, tag=f"txtT{c}", name=f"txtT{c}")
                            for c in range(NTC)]
                    for g in range(4):
                        for c in range(NTC):
                            nc.sync.dma_start(
                                out=txtT[c][:, g * QC:(g + 1) * QC],
                                in_=txt[g * QC:(g + 1) * QC, c * P:(c + 1) * P],
                                transpose=True,
                            )

                    # ---- Phase V: V[k,h] = txtT^T Wv ----------------------
                    for k in range(NKT):
                        for hc in range(HID // QC):
                            ps = psum.tile([P, QC], F32, tag="pt", bufs=2,
                                           name="pt")
                            for c in range(NTC):
                                nc.tensor.matmul(
                                    ps[:],
                                    txtT[c][:, k * P:(k + 1) * P],
                                    wv_all[:, c, hc * QC:(hc + 1) * QC],
                                    start=(c == 0),
                                    stop=(c == NTC - 1),
                                )
                            dst = v_t[k][:, hc * QC:(hc + 1) * QC]
                            if (k + hc) % 2:
                                nc.vector.tensor_copy(dst, ps[:])
                            else:
                                nc.scalar.copy(dst, ps[:])

                    # ---- Phase K: Kt[h,k] = Wk^T txtT (no bk) -------------
                    for h in range(NHT):
                        for kc in range(LK // QC):
                            ps = psum.tile([P, QC], F32, tag="pt", bufs=2,
                                           name="pt")
                            for c in range(NTC):
                                nc.tensor.matmul(
                                    ps[:],
                                    wk_all[:, c, h * P:(h + 1) * P],
                                    txtT[c][:, kc * QC:(kc + 1) * QC],
                                    start=(c == 0),
                                    stop=(c == NTC - 1),
                                )
                            dst = kt_t[h][:, kc * QC:(kc + 1) * QC]
                            if (h + kc) % 2:
                                nc.vector.tensor_copy(dst, ps[:])
                            else:
                                nc.scalar.copy(dst, ps[:])

                # ---- per-q-chunk: Qt -> S -> exp -> O -> epilogue ---------
                with tc.tile_pool(name=f"attn{rep}", bufs=1) as attn:
                    def load_imgT(qc):
                        tiles = []
                        for c in range(NIC):
                            t = attn.tile([P, QC], BF16, tag=f"imgT{c}", bufs=2,
                                          name=f"imgT{c}")
                            nc.sync.dma_start(
                                out=t[:],
                                in_=img[qc * QC:(qc + 1) * QC,
                                        c * P:(c + 1) * P],
                                transpose=True,
                            )
                            tiles.append(t)
                        return tiles

                    imgT = load_imgT(0)
                    for qc in range(NQC):
                        # Qt for this chunk
                        qt = []
                        for h in range(NHT):
                            ps = psum.tile([P, QC], F32, tag="pt", bufs=2,
                                           name="pt")
                            for c in range(NIC):
                                nc.tensor.matmul(
                                    ps[:],
                                    wq_all[:, c, h * P:(h + 1) * P],
                                    imgT[c][:],
                                    start=(c == 0),
                                    stop=(c == NIC - 1),
                                )
                            qh = attn.tile([P, QC], BF16, tag=f"qt{h}", bufs=2,
                                           name=f"qt{h}")
                            nc.vector.tensor_scalar_add(qh[:], ps[:],
                                                        bq_t[:, h:h + 1])
                            qt.append(qh)
                        # prefetch imgT for next chunk
                        if qc + 1 < NQC:
                            imgT = load_imgT(qc + 1)
                        # S + exp
                        e_t = []
                        for k in range(NKT):
                            ps = psum.tile([P, QC], F32, tag="pt", bufs=2,
                                           name="pt")
                            for h in range(NHT):
                                nc.tensor.matmul(
                                    ps[:],
                                    kt_t[h][:, k * P:(k + 1) * P],
                                    qt[h][:],
                                    start=(h == 0),
                                    stop=(h == NHT - 1),
                                )
                            e = attn.tile([P, QC], BF16, tag=f"e{k}", bufs=2,
                                          name=f"e{k}")
                            nc.scalar.activation(e[:], ps[:], AF.Exp,
                                                 scale=float(SCALE))
                            e_t.append(e)
                        # O + row sums + epilogue
                        for qs in range(QC // P):
                            po0 = psum.tile([P, QC], F32, tag="po0", bufs=2,
                                            name="po0")
                            po1 = psum.tile([P, QC], F32, tag="po1", bufs=2,
                                            name="po1")
                            pn = psum.tile([P, 2], F32, tag="pn", bufs=2,
                                           name="pn")
                            for k in range(NKT):
                                esl = e_t[k][:, qs * P:(qs + 1) * P]
                                nc.tensor.matmul(
                                    po0[:], esl, v_t[k][:, 0:QC],
                                    start=(k == 0), stop=(k == NKT - 1),
                                )
                                nc.tensor.matmul(
                                    po1[:], esl, v_t[k][:, QC:HID],
                                    start=(k == 0), stop=(k == NKT - 1),
                                )
                                nc.tensor.matmul(
                                    pn[:], esl, ones[:],
                                    start=(k == 0), stop=(k == NKT - 1),
                                )
                            rs = attn.tile([P, 1], F32, tag="rs", bufs=2,
                                           name="rs")
                            nc.vector.reciprocal(rs[:], pn[:, 0:1])
                            ot = attn.tile([P, HID], F32, tag="ot", bufs=2,
                                           name="ot")
                            row = qc * QC + qs * P
                            # halves pipelined: DVE does half 0, ACT half 1
                            nc.vector.tensor_scalar_mul(ot[:, 0:QC], po0[:],
                                                        rs[:])
                            nc.vector.tensor_add(ot[:, 0:QC], ot[:, 0:QC],
                                                 bv_bc[:, 0:QC])
                            nc.gpsimd.dma_start(out=out[row:row + P, 0:QC],
                                              in_=ot[:, 0:QC])
                            nc.scalar.activation(ot[:, QC:HID], po1[:],
                                                 AF.Copy, scale=rs[:])
                            nc.vector.tensor_add(ot[:, QC:HID], ot[:, QC:HID],
                                                 bv_bc[:, QC:HID])
                            nc.gpsimd.dma_start(out=out[row:row + P, QC:HID],
                                              in_=ot[:, QC:HID])

    nc.compile()
    return nc


def _get_nc():
    if "nc" not in _CACHED:
        _CACHED["nc"] = build_kernel()
    return _CACHED["nc"]


def kernel(image_features, text_features, Wq, bq, Wk, bk, Wv, bv):
    bf = ml_dtypes.bfloat16
    img = np.ascontiguousarray(np.asarray(image_features).astype(bf))
    txt = np.ascontiguousarray(np.asarray(text_features).astype(bf))
    shared = {
        "wq": np.ascontiguousarray(np.asarray(Wq).astype(bf)),
        "wk": np.ascontiguousarray(np.asarray(Wk).astype(bf)),
        "wv": np.ascontiguousarray(np.asarray(Wv).astype(bf)),
        "bq": np.ascontiguousarray(np.asarray(bq, np.float32)),
        "bv": np.ascontiguousarray(np.asarray(bv, np.float32)),
    }
    in_maps = [{"img": img[b], "txt": txt[b], **shared} for b in range(B)]
    res = run_bass_kernel_spmd(_get_nc(), in_maps, core_ids=list(range(B)))
    return np.stack([res.results[b]["out_attn"] for b in range(B)])


# revision 13
# speedup vs baseline: 1.0863x; 1.0863x over previous
"""CrossModalityAttention Trainium2 Bass kernel.

Data-parallel over batch: 8 cores, one batch element each.
Per core (b): out[b] = softmax((img[b]@Wq + bq) @ (txt[b]@Wk + bk)^T / 32) @ (txt[b]@Wv + bv)

Key choices vs the fp32r baseline (545us):
  * All matmul operands bf16 (host casts img/txt/W* to bf16). fp32 PSUM
    accumulation. Measured end-to-end rel err ~3e-3 << 2e-2 budget.
  * bk dropped entirely: S[q,k] = Q.K[k] + Q.bk is a per-row constant shift
    under row-softmax, so it cancels.
  * imgT / txtT produced by XBAR DMA transpose (16x128-tile crossbar, 2-byte
    dtypes) straight from DRAM -- zero PE transpose work, no identity matrix.
  * Everything SBUF-resident: txtT 3MB, V 4MB, Kt 4MB, weights 5MB, per-qc
    Qt/imgT/E double-buffered. No DRAM scratch round-trips.
  * Phase order V -> Kt -> per-q-chunk [Qt -> S -> exp -> O -> epilogue],
    emitted so the PE never waits on anything but the first ~2MB of DMA.

Layout (contraction dim always = partition dim):
  txtT[d, k] (XBAR)      imgT[i, q-chunk] (XBAR, double-buffered)
  V[k, h]    = txtT^T Wv               -> SBUF bf16 (bv folded in epilogue)
  Kt[h, k]   = Wk^T txtT               -> SBUF bf16 (no bk)
  Qt[h, q]   = Wq^T imgT + bq          -> SBUF bf16, per q-chunk
  S[k, q]    = Kt^T Qt   (psum f32)
  E = exp(S/32)          (ACT, psum -> SBUF bf16)
  O[q, h]    = E^T V, sums[q] = E^T ones, out = O/sums + bv
"""

import numpy as np
import ml_dtypes

import concourse.bass as bass
import concourse.tile as tile
from concourse import bacc, mybir
from concourse.bass_utils import run_bass_kernel_spmd

F32 = mybir.dt.float32
BF16 = mybir.dt.bfloat16
FP8 = mybir.dt.float8e4
AF = mybir.ActivationFunctionType
DR = mybir.MatmulPerfMode.DoubleRow

# S = Kt^T Qt in fp8e4 DoubleRow (2 contraction subtiles per pass, 0.5
# cycles/row): ~2x the S-matmul throughput for ~1.4e-2 rel err (budget 2e-2,
# deterministic inputs). Flip to False for the all-bf16 (~3e-3) variant.
FP8_S = True

P = 128
B, LQ, LK = 8, 2048, 2048
IMG, TXT, HID = 1024, 768, 1024
NKT = LK // P                 # 16 key tiles
NTC = TXT // P                # 6 txt contraction chunks
NIC = IMG // P                # 8 img contraction chunks
NHT = HID // P                # 8 hid tiles
QC = 512                      # q chunk width
NQC = LQ // QC                # 4
SCALE = 1.0 / np.sqrt(np.float32(HID))

_CACHED = {}


def build_kernel(reps=1):
    nc = bacc.Bacc("TRN2", target_bir_lowering=False, debug=False)
    img = nc.dram_tensor("img", [LQ, IMG], BF16, kind="ExternalInput").ap()
    txt = nc.dram_tensor("txt", [LK, TXT], BF16, kind="ExternalInput").ap()
    wq = nc.dram_tensor("wq", [IMG, HID], BF16, kind="ExternalInput").ap()
    wk = nc.dram_tensor("wk", [TXT, HID], BF16, kind="ExternalInput").ap()
    wv = nc.dram_tensor("wv", [TXT, HID], BF16, kind="ExternalInput").ap()
    bq = nc.dram_tensor("bq", [HID], F32, kind="ExternalInput").ap()
    bv = nc.dram_tensor("bv", [HID], F32, kind="ExternalInput").ap()
    out = nc.dram_tensor("out_attn", [LQ, HID], F32, kind="ExternalOutput").ap()

    with tile.TileContext(nc) as tc:
        with (
            tc.tile_pool(name="sb", bufs=1) as sb,
            tc.tile_pool(name="psum", bufs=1, space="PSUM") as psum,
        ):
            ones = sb.tile([P, 2], BF16, tag="ones")
            nc.vector.memset(ones[:], 1.0)
            bq_t = sb.tile([P, NHT], F32, tag="bq")
            nc.gpsimd.dma_start(out=bq_t[:], in_=bq.rearrange("(t p) -> p t", p=P))
            bv_bc = sb.tile([P, HID], F32, tag="bv")
            nc.gpsimd.dma_start(out=bv_bc[:], in_=bv.partition_broadcast(P))

            v_t = [sb.tile([P, HID], BF16, tag=f"v{k}", name=f"v{k}")
                   for k in range(NKT)]
            if FP8_S:
                # [h-partition, h-tile, k] so DoubleRow can take 2 h-tiles
                # per matmul via [:, s:s+2, kslice]
                kt8 = sb.tile([P, NHT, LK], FP8, tag="kt8", name="kt8")
            else:
                kt_t = [sb.tile([P, LK], BF16, tag=f"kt{h}", name=f"kt{h}")
                        for h in range(NHT)]

            for rep in range(reps):
                with tc.tile_pool(name=f"proj{rep}", bufs=1) as proj:
                    # ---- weight loads on ACT hwdge queue (parallel to SP) --
                    # wv split in h-halves so V(k=0,hc=0) gates on 0.75MB only
                    wv_all = proj.tile([P, NTC, HID], BF16, tag="wv", name="wv")
                    wv_r = wv.rearrange("(c p) h -> p c h", p=P)
                    nc.scalar.dma_start(out=wv_all[:, :, 0:QC],
                                        in_=wv_r[:, :, 0:QC])
                    nc.scalar.dma_start(out=wv_all[:, :, QC:HID],
                                        in_=wv_r[:, :, QC:HID])
                    wk_all = proj.tile([P, NTC, HID], BF16, tag="wk", name="wk")
                    nc.scalar.dma_start(out=wk_all[:],
                                        in_=wk.rearrange("(c p) h -> p c h", p=P))
                    wq_all = sb.tile([P, NIC, HID], BF16, tag="wq", name="wq")
                    nc.scalar.dma_start(out=wq_all[:],
                                        in_=wq.rearrange("(c p) h -> p c h", p=P))

                    # ---- txtT via XBAR transpose on SP queue --------------
                    txtT = [proj.tile([P, LK], BF16, tag=f"txtT{c}", name=f"txtT{c}")
                            for c in range(NTC)]
                    for g in range(4):
                        for c in range(NTC):
                            nc.sync.dma_start(
                                out=txtT[c][:, g * QC:(g + 1) * QC],
                                in_=txt[g * QC:(g + 1) * QC, c * P:(c + 1) * P],
                                transpose=True,
                            )

                    # ---- Phase V: V[k,h] = txtT^T Wv ----------------------
                    for k in range(NKT):
                        for hc in range(HID // QC):
                            ps = psum.tile([P, QC], F32, tag="pt", bufs=2,
                                           name="pt")
                            for c in range(NTC):
                                nc.tensor.matmul(
                                    ps[:],
                                    txtT[c][:, k * P:(k + 1) * P],
                                    wv_all[:, c, hc * QC:(hc + 1) * QC],
                                    start=(c == 0),
                                    stop=(c == NTC - 1),
                                )
                            dst = v_t[k][:, hc * QC:(hc + 1) * QC]
                            if (k + hc) % 2:
                                nc.vector.tensor_copy(dst, ps[:])
                            else:
                                nc.scalar.copy(dst, ps[:])

                    # ---- Phase K: Kt[h,k] = Wk^T txtT (no bk) -------------
                    for h in range(NHT):
                        for kc in range(LK // QC):
                            ps = psum.tile([P, QC], F32, tag="pt", bufs=2,
                                           name="pt")
                            for c in range(NTC):
                                nc.tensor.matmul(
                                    ps[:],
                                    wk_all[:, c, h * P:(h + 1) * P],
                                    txtT[c][:, kc * QC:(kc + 1) * QC],
                                    start=(c == 0),
                                    stop=(c == NTC - 1),
                                )
                            if FP8_S:
                                dst = kt8[:, h, kc * QC:(kc + 1) * QC]
                            else:
                                dst = kt_t[h][:, kc * QC:(kc + 1) * QC]
                            if (h + kc) % 2:
                                nc.vector.tensor_copy(dst, ps[:])
                            else:
                                nc.scalar.copy(dst, ps[:])

                # ---- per-q-chunk: Qt -> S -> exp -> O -> epilogue ---------
                with tc.tile_pool(name=f"attn{rep}", bufs=1) as attn:
                    def load_imgT(qc):
                        tiles = []
                        for c in range(NIC):
                            t = attn.tile([P, QC], BF16, tag=f"imgT{c}", bufs=2,
                                          name=f"imgT{c}")
                            nc.sync.dma_start(
                                out=t[:],
                                in_=img[qc * QC:(qc + 1) * QC,
                                        c * P:(c + 1) * P],
                                transpose=True,
                            )
                            tiles.append(t)
                        return tiles

                    imgT = load_imgT(0)
                    for qc in range(NQC):
                        # Qt for this chunk
                        if FP8_S:
                            qt8 = attn.tile([P, NHT, QC], FP8, tag="qt8",
                                            bufs=2, name="qt8")
                        else:
                            qt = []
                        for h in range(NHT):
                            ps = psum.tile([P, QC], F32, tag="pt", bufs=2,
                                           name="pt")
                            for c in range(NIC):
                                nc.tensor.matmul(
                                    ps[:],
                                    wq_all[:, c, h * P:(h + 1) * P],
                                    imgT[c][:],
                                    start=(c == 0),
                                    stop=(c == NIC - 1),
                                )
                            if FP8_S:
                                nc.vector.tensor_scalar_add(qt8[:, h, :], ps[:],
                                                            bq_t[:, h:h + 1])
                            else:
                                qh = attn.tile([P, QC], BF16, tag=f"qt{h}",
                                               bufs=2, name=f"qt{h}")
                                nc.vector.tensor_scalar_add(qh[:], ps[:],
                                                            bq_t[:, h:h + 1])
                                qt.append(qh)
                        # prefetch imgT for next chunk
                        if qc + 1 < NQC:
                            imgT = load_imgT(qc + 1)
                        # S + exp
                        e_t = []
                        for k in range(NKT):
                            ps = psum.tile([P, QC], F32, tag="pt", bufs=2,
                                           name="pt")
                            if FP8_S:
                                for s in range(0, NHT, 2):
                                    nc.tensor.matmul(
                                        ps[:],
                                        kt8[:, s:s + 2, k * P:(k + 1) * P],
                                        qt8[:, s:s + 2, :],
                                        start=(s == 0),
                                        stop=(s == NHT - 2),
                                        perf_mode=DR,
                                    )
                            else:
                                for h in range(NHT):
                                    nc.tensor.matmul(
                                        ps[:],
                                        kt_t[h][:, k * P:(k + 1) * P],
                                        qt[h][:],
                                        start=(h == 0),
                                        stop=(h == NHT - 1),
                                    )
                            e = attn.tile([P, QC], BF16, tag=f"e{k}", bufs=2,
                                          name=f"e{k}")
                            nc.scalar.activation(e[:], ps[:], AF.Exp,
                                                 scale=float(SCALE))
                            e_t.append(e)
                        # O + row sums + epilogue
                        for qs in range(QC // P):
                            po0 = psum.tile([P, QC], F32, tag="po0", bufs=2,
                                            name="po0")
                            po1 = psum.tile([P, QC], F32, tag="po1", bufs=2,
                                            name="po1")
                            pn = psum.tile([P, 2], F32, tag="pn", bufs=2,
                                           name="pn")
                            for k in range(NKT):
                                esl = e_t[k][:, qs * P:(qs + 1) * P]
                                nc.tensor.matmul(
                                    po0[:], esl, v_t[k][:, 0:QC],
                                    start=(k == 0), stop=(k == NKT - 1),
                                )
                                nc.tensor.matmul(
                                    po1[:], esl, v_t[k][:, QC:HID],
                                    start=(k == 0), stop=(k == NKT - 1),
                                )
                                nc.tensor.matmul(
                                    pn[:], esl, ones[:],
                                    start=(k == 0), stop=(k == NKT - 1),
                                )
                            rs = attn.tile([P, 1], F32, tag="rs", bufs=2,
                                           name="rs")
                            nc.vector.reciprocal(rs[:], pn[:, 0:1])
                            ot = attn.tile([P, HID], F32, tag="ot", bufs=2,
                                           name="ot")
                            row = qc * QC + qs * P
                            # halves pipelined on DVE, stores on ACT queue
                            nc.vector.tensor_scalar_mul(ot[:, 0:QC], po0[:],
                                                        rs[:])
                            nc.vector.tensor_add(ot[:, 0:QC], ot[:, 0:QC],
                                                 bv_bc[:, 0:QC])
                            nc.scalar.dma_start(out=out[row:row + P, 0:QC],
                                                in_=ot[:, 0:QC])
                            nc.vector.tensor_scalar_mul(ot[:, QC:HID], po1[:],
                                                        rs[:])
                            nc.vector.tensor_add(ot[:, QC:HID], ot[:, QC:HID],
                                                 bv_bc[:, QC:HID])
                            nc.scalar.dma_start(out=out[row:row + P, QC:HID],
                                                in_=ot[:, QC:HID])

    nc.compile()
    return nc


def _get_nc():
    if "nc" not in _CACHED:
        _CACHED["nc"] = build_kernel()
    return _CACHED["nc"]


def kernel(image_features, text_features, Wq, bq, Wk, bk, Wv, bv):
    bf = ml_dtypes.bfloat16
    img = np.ascontiguousarray(np.asarray(image_features).astype(bf))
    txt = np.ascontiguousarray(np.asarray(text_features).astype(bf))
    shared = {
        "wq": np.ascontiguousarray(np.asarray(Wq).astype(bf)),
        "wk": np.ascontiguousarray(np.asarray(Wk).astype(bf)),
        "wv": np.ascontiguousarray(np.asarray(Wv).astype(bf)),
        "bq": np.ascontiguousarray(np.asarray(bq, np.float32)),
        "bv": np.ascontiguousarray(np.asarray(bv, np.float32)),
    }
    in_maps = [{"img": img[b], "txt": txt[b], **shared} for b in range(B)]
    res = run_bass_kernel_spmd(_get_nc(), in_maps, core_ids=list(range(B)))
    return np.stack([res.results[b]["out_attn"] for b in range(B)])
